# revision 3
# baseline (speedup 1.0000x reference)
"""BertMultiPooler (segment_reduce) Trainium2 Bass kernel, v2.

out[b*K+k] = tanh( segmean(hidden[b], seg k) @ Wd.T + bd
                   + hidden[b, pos[b,k]] @ Wt.T + bt )

Strategy (data-parallel over batch, 8 cores x 4 rows). The cost model's
binding constraint is the serial per-core DMA stream (~360 B/ns), so the
kernel is organized to keep that stream saturated end to end and to
minimize total bytes moved:
  - hidden streamed fp32 via HWDGE directly into float32r tiles: the PE
    consumes fp32r at 1 cycle/column (output free size >= 256), so the
    fp32->fp16 cast stage of v1 (83us of ACT time + pipeline stalls) is
    gone entirely.
  - Segment sums via one-hot membership matmul: for each 128-token tile,
    M[t, k] = [t >= s_k] - [t >= s_{k+1}] built on DVE in fp32r, then
    PE-matmul M.T @ hidden_tile accumulating into PSUM [64, 768].
  - Weights are cast to fp16 on the host (halves their DMA bytes); the
    combined bias row enters the dense PSUM accumulation via a
    ones-vector matmul, removing the bias DMA + DVE add.
  - CLS rows gathered with indirect DMA up front; their transposes and
    the W_tab dense matmuls run during the bulk stream, so the per-row
    tail is only: segment-mean scale, 6 transposes, 6 W_dense matmuls,
    tanh, store. The last row's final chunk is split small to shorten
    the critical tail after the last hidden byte lands.
"""

import numpy as np
from contextlib import ExitStack

import concourse.bass as bass
import concourse.bacc as bacc
import concourse.tile as tile
from concourse import mybir
from concourse.bass_utils import run_bass_kernel_spmd
from concourse.masks import make_identity

B, S, H, K = 32, 4096, 768, 64
NCORES = 8
RPC = B // NCORES  # batch rows per core
P = 128
HT = H // P        # 6 h-tiles
F32 = mybir.dt.float32
F32R = mybir.dt.float32r
F16 = mybir.dt.float16
I32 = mybir.dt.int32
OP = mybir.AluOpType


def build_nc(s=S, rpc=RPC, chunk=8, hbufs=5, rows_used=None, repeat=1,
             taper=(4, 2, 1, 1)):
    """Build the per-core Bass module. Each core gets `rpc` batch rows of
    `s` tokens each. rows_used (for benching): only process that many rows."""
    tt = s // P  # token tiles per row
    assert tt % chunk == 0
    if rows_used is None:
        rows_used = rpc

    nc = bacc.Bacc("TRN2", target_bir_lowering=False, debug=False)

    # hidden is declared float32r (same bits as the fp32 input) so the PE
    # can consume the DMA-written tiles in fast-fp32 mode directly.
    hid = nc.dram_tensor("hid", [rpc * s, H], F32R, kind="ExternalInput")
    # sxs[r*(K+1)+k] = min(pos[r,k], L) for k < K, = L for k = K
    sxs = nc.dram_tensor("sxs", [1, rpc * (K + 1)], F32, kind="ExternalInput")
    icnt = nc.dram_tensor("icnt", [K, rpc, 1], F32, kind="ExternalInput")
    gidx = nc.dram_tensor("gidx", [K, rpc, 1], I32, kind="ExternalInput")
    wdt = nc.dram_tensor("wdt", [H, H], F16, kind="ExternalInput")  # W_dense.T
    wtt = nc.dram_tensor("wtt", [H, H], F16, kind="ExternalInput")  # W_tab.T
    bia = nc.dram_tensor("bia", [1, H], F32, kind="ExternalInput")  # bd+bt
    iot = nc.dram_tensor("iot", [P, tt], F32, kind="ExternalInput")  # iot[p,i]=p+128*i
    out = nc.dram_tensor("out", [rpc, K, H], F32, kind="ExternalOutput")

    with tile.TileContext(nc) as tc:
        with ExitStack() as ctx:
            cpool = ctx.enter_context(tc.tile_pool(name="const", bufs=1))
            hpool = ctx.enter_context(tc.tile_pool(name="hpool", bufs=hbufs))
            mpool = ctx.enter_context(tc.tile_pool(name="mpool", bufs=12))
            spool = ctx.enter_context(tc.tile_pool(name="spool", bufs=2))
            tpool = ctx.enter_context(tc.tile_pool(name="tpool", bufs=2))
            # PSUM banks: pseg 1x2 + pout 2x2 + ptr 2x1 = 8 exactly. pseg
            # bufs=1 is safe: the next row's first matmul sits behind this
            # row's whole tail in the in-order PE queue, so the segment-mean
            # read has long drained the bank by then.
            pseg_pool = ctx.enter_context(
                tc.tile_pool(name="pseg", bufs=1, space="PSUM")
            )
            pout_pool = ctx.enter_context(
                tc.tile_pool(name="pout", bufs=2, space="PSUM")
            )
            ptr_pool = ctx.enter_context(tc.tile_pool(name="ptr", bufs=2, space="PSUM"))

            # DMA-engine grants are FIFO by descriptor-ready time, so issue
            # order here is a schedule: gidx rides the scalar queue first (the
            # CLS gathers' prep blocks on its data), the gathers enter the
            # FIFO next, and the small consts sit on the sync queue between
            # the weights and the hidden chunks — their descriptor-gen time
            # delays the chunks' FIFO entry just enough for the gathers to
            # land before the third 17us chunk hold.
            gidx_t = cpool.tile([K, rpc, 1], I32)
            nc.scalar.dma_start(gidx_t[:], gidx.ap())
            tabs = cpool.tile([K, rpc, H], F16)
            for r in range(rpc):
                nc.gpsimd.indirect_dma_start(
                    out=tabs[:, r, :],
                    out_offset=None,
                    in_=hid.ap(),
                    in_offset=bass.IndirectOffsetOnAxis(ap=gidx_t[:, r, :], axis=0),
                )

            wdt_t = cpool.tile([P, HT, H], F16)
            nc.sync.dma_start(wdt_t[:], wdt.ap().rearrange("(j p) h -> p j h", p=P))
            wtt_t = cpool.tile([P, HT, H], F16)
            nc.sync.dma_start(wtt_t[:], wtt.ap().rearrange("(j p) h -> p j h", p=P))
            sx1 = cpool.tile([1, rpc * (K + 1)], F32)
            nc.sync.dma_start(sx1[:], sxs.ap())
            b32 = cpool.tile([1, H], F32)
            nc.sync.dma_start(b32[:], bia.ap())
            iota_t = cpool.tile([P, tt], F32)
            nc.sync.dma_start(iota_t[:], iot.ap())
            icnt_t = cpool.tile([K, rpc, 1], F32)
            nc.sync.dma_start(icnt_t[:], icnt.ap())

            # ---- constants ----
            id16 = cpool.tile([P, P], F16)
            make_identity(nc, id16[:])
            ones_k = cpool.tile([1, K], F16)
            nc.vector.memset(ones_k[:], 1.0)
            ones_p = cpool.tile([1, P], F32)
            nc.vector.memset(ones_p[:], 1.0)
            b16 = cpool.tile([1, H], F16)
            nc.vector.tensor_copy(b16[:], b32[:])

            # broadcast segment boundaries to all 128 partitions via PE
            psxb = ptr_pool.tile([P, rpc * (K + 1)], F32, tag="ptr")
            nc.tensor.matmul(psxb[:], ones_p[:], sx1[:], start=True, stop=True)
            sxb = cpool.tile([P, rpc, K + 1], F32)
            nc.vector.tensor_copy(sxb[:], psxb[:].rearrange("p (r k) -> p r k", r=rpc))

            hid_v = hid.ap().rearrange("(r n p) h -> p r n h", r=rpc, p=P)

            row_seq = [r for _ in range(repeat) for r in range(rows_used)]
            for ridx, r in enumerate(row_seq):
                last = ridx == len(row_seq) - 1

                # ---- early dense work: bias + tab @ Wt.T into pout PSUM ----
                pout = pout_pool.tile([K, H], F32)
                nc.tensor.matmul(
                    pout[:, 0:512], ones_k[:], b16[:, 0:512], start=True, stop=False
                )
                nc.tensor.matmul(
                    pout[:, 512:H], ones_k[:], b16[:, 512:H], start=True, stop=False
                )
                xTt = tpool.tile([P, HT, K], F16, tag="xTt")
                for j in range(HT):
                    ptr1 = ptr_pool.tile([P, K], F16, tag="ptr")
                    nc.tensor.transpose(
                        out=ptr1[:],
                        in_=tabs[:, r, j * P : (j + 1) * P],
                        identity=id16[0:K, 0:K],
                    )
                    nc.vector.tensor_copy(xTt[:, j, :], ptr1[:])
                    nc.tensor.matmul(
                        pout[:, 0:512], xTt[:, j, :], wtt_t[:, j, 0:512],
                        start=False, stop=False,
                    )
                    nc.tensor.matmul(
                        pout[:, 512:H], xTt[:, j, :], wtt_t[:, j, 512:H],
                        start=False, stop=False,
                    )

                # ---- segment sums into PSUM [K, H] ----
                pseg = pseg_pool.tile([K, H], F32)
                schedule = [chunk] * (tt // chunk)
                if last and taper:
                    # taper the final transfers so the closing tail starts
                    # after a short hold and the PE keeps pace with the DMA
                    ntap = sum(taper)
                    assert ntap % chunk == 0
                    schedule = schedule[: -(ntap // chunk)] + list(taper)
                t0 = 0
                for nch in schedule:
                    hbuf = hpool.tile([P, chunk, H], F32R, tag="hbuf")
                    nc.sync.dma_start(hbuf[:, 0:nch, :], hid_v[:, r, t0 : t0 + nch, :])
                    for i in range(nch):
                        t = t0 + i
                        ge = mpool.tile([P, K + 1], F32, tag="ge")
                        nc.vector.tensor_scalar(
                            ge[:],
                            sxb[:, r, :],
                            iota_t[:, t : t + 1],
                            None,
                            OP.is_le,
                        )
                        m01 = mpool.tile([P, K], F32R, tag="m01")
                        nc.vector.tensor_tensor(
                            out=m01[:],
                            in0=ge[:, 0:K],
                            in1=ge[:, 1 : K + 1],
                            op=OP.subtract,
                        )
                        nc.tensor.matmul(
                            pseg[:, 0:512],
                            m01[:],
                            hbuf[:, i, 0:512],
                            start=(t == 0),
                            stop=(t == tt - 1),
                        )
                        nc.tensor.matmul(
                            pseg[:, 512:H],
                            m01[:],
                            hbuf[:, i, 512:H],
                            start=(t == 0),
                            stop=(t == tt - 1),
                        )
                    t0 += nch

                # ---- tail: segment mean, transpose, W_dense, tanh, store ----
                segs = spool.tile([K, H], F16, tag="segs")
                # two halves so the first transposes start half a scale early
                nc.vector.tensor_scalar(
                    segs[:, 0 : H // 2], pseg[:, 0 : H // 2], icnt_t[:, r, :],
                    None, OP.mult,
                )
                nc.vector.tensor_scalar(
                    segs[:, H // 2 : H], pseg[:, H // 2 : H], icnt_t[:, r, :],
                    None, OP.mult,
                )
                for j in range(HT):
                    ptr2 = ptr_pool.tile([P, K], F16, tag="ptr")
                    nc.tensor.transpose(
                        out=ptr2[:],
                        in_=segs[:, j * P : (j + 1) * P],
                        identity=id16[0:K, 0:K],
                    )
                    xTs = tpool.tile([P, K], F16, tag=f"xTs{j}")
                    nc.vector.tensor_copy(xTs[:], ptr2[:])
                    nc.tensor.matmul(
                        pout[:, 0:512], xTs[:], wdt_t[:, j, 0:512],
                        start=False, stop=(j == HT - 1),
                    )
                    nc.tensor.matmul(
                        pout[:, 512:H], xTs[:], wdt_t[:, j, 512:H],
                        start=False, stop=(j == HT - 1),
                    )

                fin = spool.tile([K, H], F32, tag="fin")
                nc.scalar.activation(
                    out=fin[:],
                    in_=pout[:],
                    func=mybir.ActivationFunctionType.Tanh,
                )
                nc.scalar.dma_start(out.ap()[r], fin[:])

    nc.compile()
    return nc


def prep_inputs(hidden_states, W_dense, b_dense, W_tab, b_tab, cls_indexes,
                table_length, s=S, rpc=RPC, ncores=NCORES):
    """Host-side index prep + per-core sharding. Returns in_maps."""
    hs = np.ascontiguousarray(np.asarray(hidden_states, dtype=np.float32))
    b = hs.shape[0]
    pos = np.asarray(cls_indexes)[:, 1].reshape(b, K).astype(np.int64)
    L = np.asarray(table_length).astype(np.int64)
    tt = s // P

    # sx[b, k] = min(pos_k, L) for k < K; sx[b, K] = L
    sx_all = np.minimum(pos, L[:, None]).astype(np.float32)
    sx_all = np.concatenate([sx_all, L[:, None].astype(np.float32)], axis=1)  # [b, K+1]
    cnt = sx_all[:, 1:] - sx_all[:, :-1]
    inv_cnt = np.where(cnt > 0, 1.0 / np.maximum(cnt, 1.0), 0.0).astype(np.float32)

    wdt = np.ascontiguousarray(np.asarray(W_dense, dtype=np.float32).T.astype(np.float16))
    wtt = np.ascontiguousarray(np.asarray(W_tab, dtype=np.float32).T.astype(np.float16))
    bia = np.ascontiguousarray(
        (np.asarray(b_dense, dtype=np.float32)
         + np.asarray(b_tab, dtype=np.float32))[None, :]
    )
    iot = (np.arange(P, dtype=np.float32)[:, None]
           + P * np.arange(tt, dtype=np.float32)[None, :])
    iot = np.ascontiguousarray(iot)

    in_maps = []
    for c in range(ncores):
        sxs_c = np.ascontiguousarray(
            sx_all[c * rpc:(c + 1) * rpc].reshape(1, rpc * (K + 1))
        )
        icnt_c = np.ascontiguousarray(
            inv_cnt[c * rpc:(c + 1) * rpc, :, None].transpose(1, 0, 2)
        )
        gidx_c = np.ascontiguousarray(
            (pos[c * rpc:(c + 1) * rpc] + (np.arange(rpc) * s)[:, None])
            .astype(np.int32)[:, :, None].transpose(1, 0, 2)
        )
        in_maps.append({
            "hid": hs[c * rpc:(c + 1) * rpc].reshape(rpc * s, H),
            "sxs": sxs_c,
            "icnt": icnt_c,
            "gidx": gidx_c,
            "wdt": wdt,
            "wtt": wtt,
            "bia": bia,
            "iot": iot,
        })
    return in_maps


_NC_CACHE = {}


def _get_nc():
    if "nc" not in _NC_CACHE:
        _NC_CACHE["nc"] = build_nc()
    return _NC_CACHE["nc"]


def run(inputs, trace=False):
    """Run on 8 cores; returns (full_output, BassKernelResults)."""
    import os

    nc = _get_nc()
    in_maps = prep_inputs(**inputs)
    prev = os.environ.get("BASS_NEVER_TRACE")
    if not trace:
        os.environ["BASS_NEVER_TRACE"] = "1"
    try:
        res = run_bass_kernel_spmd(
            nc, in_maps, core_ids=list(range(NCORES)), trace=trace
        )
    finally:
        if not trace:
            if prev is None:
                os.environ.pop("BASS_NEVER_TRACE", None)
            else:
                os.environ["BASS_NEVER_TRACE"] = prev
    outs = [res.results[c]["out"].reshape(RPC * K, H) for c in range(NCORES)]
    return np.concatenate(outs, axis=0), res


def kernel(**inputs) -> np.ndarray:
    out, _ = run(inputs, trace=False)
    return out


def bench(inputs, iters=20):
    """Time the on-device NEFF execution: inputs staged to the 8 devices
    once, then `iters` pipelined executes. Returns (output, secs_per_iter)."""
    nc = _get_nc()
    in_maps = prep_inputs(**inputs)
    rets, dt, dt_ser = pjrt_bench(nc, in_maps, iters)
    final = np.asarray(rets[0]).reshape(NCORES, RPC * K, H).reshape(B * K, H)
    return final, dt, dt_ser


def pjrt_bench(nc, in_maps, iters=20, ncores=NCORES):
    """Generic: jit+shard a Bass module on `ncores` devices, stage inputs,
    time pipelined and serialized executes. Returns (concat_outs, dt, dt_ser)."""
    rets, timeit = make_runner(nc, in_maps, ncores)
    dt = min(timeit(iters) for _ in range(3))
    dt_ser = dt
    return rets, dt, dt_ser


def make_runner(nc, in_maps, ncores=NCORES):
    """Stage a Bass module + inputs on the devices; return (outputs,
    timeit(iters) -> secs/iter for pipelined executes)."""
    import time

    import jax
    from jax.sharding import Mesh, NamedSharding, PartitionSpec
    from jax.experimental.shard_map import shard_map

    from concourse import bass2jax

    bass2jax.install_neuronx_cc_hook()

    partition_name = nc.partition_id_tensor.name if nc.partition_id_tensor else None
    in_names, out_names, out_avals = [], [], []
    for alloc in nc.m.functions[0].allocations:
        if not isinstance(alloc, mybir.MemoryLocationSet):
            continue
        name = alloc.memorylocations[0].name
        if alloc.kind == "ExternalInput":
            if name != partition_name:
                in_names.append(name)
        elif alloc.kind == "ExternalOutput":
            out_names.append(name)
            out_avals.append(
                jax.core.ShapedArray(
                    tuple(alloc.tensor_shape), mybir.dt.np(alloc.dtype)
                )
            )
    n_params = len(in_names)
    all_names = tuple(in_names) + tuple(out_names)
    if partition_name is not None:
        all_names = all_names + (partition_name,)

    def _body(*args):
        operands = list(args)
        if partition_name is not None:
            operands.append(bass2jax.partition_id_tensor())
        outs = bass2jax._bass_exec_p.bind(
            *operands,
            out_avals=tuple(out_avals),
            in_names=all_names,
            out_names=tuple(out_names),
            lowering_input_output_aliases=(),
            sim_require_finite=True,
            sim_require_nnan=True,
            nc=nc,
        )
        return tuple(outs)

    devices = jax.devices()[:ncores]
    mesh = Mesh(np.asarray(devices), ("core",))
    spec = PartitionSpec("core")
    nspecs = n_params + len(out_names)
    sharded = jax.jit(
        shard_map(
            _body,
            mesh=mesh,
            in_specs=(spec,) * nspecs,
            out_specs=(spec,) * len(out_names),
            check_rep=False,
        ),
        keep_unused=True,
    )
    sh = NamedSharding(mesh, spec)
    concat_in = [
        jax.device_put(
            np.concatenate([np.asarray(in_maps[c][n]) for c in range(ncores)], 0), sh
        )
        for n in in_names
    ]
    concat_zero = [
        jax.device_put(
            np.zeros((ncores * a.shape[0], *a.shape[1:]), a.dtype), sh
        )
        for a in out_avals
    ]

    out = sharded(*concat_in, *concat_zero)
    jax.block_until_ready(out)

    def timeit(iters):
        t0 = time.perf_counter()
        rets = [sharded(*concat_in, *concat_zero) for _ in range(iters)]
        jax.block_until_ready(rets)
        return (time.perf_counter() - t0) / iters

    return out, timeit



# revision 4
# speedup vs baseline: 1.0032x; 1.0032x over previous
"""BertMultiPooler (segment_reduce) Trainium2 Bass kernel, v2.

out[b*K+k] = tanh( segmean(hidden[b], seg k) @ Wd.T + bd
                   + hidden[b, pos[b,k]] @ Wt.T + bt )

Strategy (data-parallel over batch, 8 cores x 4 rows). The cost model's
binding constraint is the serial per-core DMA stream (~360 B/ns), so the
kernel is organized to keep that stream saturated end to end and to
minimize total bytes moved:
  - hidden streamed fp32 via HWDGE directly into float32r tiles: the PE
    consumes fp32r at 1 cycle/column (output free size >= 256), so the
    fp32->fp16 cast stage of v1 (83us of ACT time + pipeline stalls) is
    gone entirely.
  - Segment sums via one-hot membership matmul: for each 128-token tile,
    M[t, k] = [t >= s_k] - [t >= s_{k+1}] built on DVE in fp32r, then
    PE-matmul M.T @ hidden_tile accumulating into PSUM [64, 768].
  - Weights are cast to fp16 on the host (halves their DMA bytes); the
    combined bias row enters the dense PSUM accumulation via a
    ones-vector matmul, removing the bias DMA + DVE add.
  - CLS rows gathered with indirect DMA up front; their transposes and
    the W_tab dense matmuls run during the bulk stream, so the per-row
    tail is only: segment-mean scale, 6 transposes, 6 W_dense matmuls,
    tanh, store. The last row's final chunk is split small to shorten
    the critical tail after the last hidden byte lands.
"""

import numpy as np
from contextlib import ExitStack

import concourse.bass as bass
import concourse.bacc as bacc
import concourse.tile as tile
from concourse import mybir
from concourse.bass_utils import run_bass_kernel_spmd
from concourse.masks import make_identity

B, S, H, K = 32, 4096, 768, 64
NCORES = 8
RPC = B // NCORES  # batch rows per core
P = 128
HT = H // P        # 6 h-tiles
F32 = mybir.dt.float32
F32R = mybir.dt.float32r
F16 = mybir.dt.float16
I32 = mybir.dt.int32
OP = mybir.AluOpType


def build_nc(s=S, rpc=RPC, chunk=8, hbufs=5, rows_used=None, repeat=1,
             taper=(4, 2, 1, 1)):
    """Build the per-core Bass module. Each core gets `rpc` batch rows of
    `s` tokens each. rows_used (for benching): only process that many rows."""
    tt = s // P  # token tiles per row
    assert tt % chunk == 0
    if rows_used is None:
        rows_used = rpc

    nc = bacc.Bacc("TRN2", target_bir_lowering=False, debug=False)

    # hidden is declared float32r (same bits as the fp32 input) so the PE
    # can consume the DMA-written tiles in fast-fp32 mode directly.
    hid = nc.dram_tensor("hid", [rpc * s, H], F32R, kind="ExternalInput")
    # sxs[r*(K+1)+k] = min(pos[r,k], L) for k < K, = L for k = K
    sxs = nc.dram_tensor("sxs", [1, rpc * (K + 1)], F32, kind="ExternalInput")
    icnt = nc.dram_tensor("icnt", [K, rpc, 1], F32, kind="ExternalInput")
    gidx = nc.dram_tensor("gidx", [K, rpc, 1], I32, kind="ExternalInput")
    wdt = nc.dram_tensor("wdt", [H, H], F16, kind="ExternalInput")  # W_dense.T
    wtt = nc.dram_tensor("wtt", [H, H], F16, kind="ExternalInput")  # W_tab.T
    bia = nc.dram_tensor("bia", [1, H], F32, kind="ExternalInput")  # bd+bt
    iot = nc.dram_tensor("iot", [P, tt], F32, kind="ExternalInput")  # iot[p,i]=p+128*i
    out = nc.dram_tensor("out", [rpc, K, H], F32, kind="ExternalOutput")

    with tile.TileContext(nc) as tc:
        with ExitStack() as ctx:
            cpool = ctx.enter_context(tc.tile_pool(name="const", bufs=1))
            hpool = ctx.enter_context(tc.tile_pool(name="hpool", bufs=hbufs))
            mpool = ctx.enter_context(tc.tile_pool(name="mpool", bufs=12))
            spool = ctx.enter_context(tc.tile_pool(name="spool", bufs=2))
            tpool = ctx.enter_context(tc.tile_pool(name="tpool", bufs=2))
            # PSUM banks: pseg 1x2 + pout 2x2 + ptr 2x1 = 8 exactly. pseg
            # bufs=1 is safe: the next row's first matmul sits behind this
            # row's whole tail in the in-order PE queue, so the segment-mean
            # read has long drained the bank by then.
            pseg_pool = ctx.enter_context(
                tc.tile_pool(name="pseg", bufs=1, space="PSUM")
            )
            pout_pool = ctx.enter_context(
                tc.tile_pool(name="pout", bufs=2, space="PSUM")
            )
            ptr_pool = ctx.enter_context(tc.tile_pool(name="ptr", bufs=2, space="PSUM"))

            # DMA-engine grants are FIFO by descriptor-ready time, so issue
            # order here is a schedule: gidx rides the scalar queue first (the
            # CLS gathers' prep blocks on its data), the gathers enter the
            # FIFO next, and the small consts sit on the sync queue between
            # the weights and the hidden chunks — their descriptor-gen time
            # delays the chunks' FIFO entry just enough for the gathers to
            # land before the third 17us chunk hold.
            gidx_t = cpool.tile([K, rpc, 1], I32)
            nc.scalar.dma_start(gidx_t[:], gidx.ap())
            tabs = cpool.tile([K, rpc, H], F16)
            for r in range(rpc):
                nc.gpsimd.indirect_dma_start(
                    out=tabs[:, r, :],
                    out_offset=None,
                    in_=hid.ap(),
                    in_offset=bass.IndirectOffsetOnAxis(ap=gidx_t[:, r, :], axis=0),
                )

            wdt_t = cpool.tile([P, HT, H], F16)
            nc.sync.dma_start(wdt_t[:], wdt.ap().rearrange("(j p) h -> p j h", p=P))
            wtt_t = cpool.tile([P, HT, H], F16)
            nc.sync.dma_start(wtt_t[:], wtt.ap().rearrange("(j p) h -> p j h", p=P))
            sx1 = cpool.tile([1, rpc * (K + 1)], F32)
            nc.sync.dma_start(sx1[:], sxs.ap())
            b32 = cpool.tile([1, H], F32)
            nc.sync.dma_start(b32[:], bia.ap())
            iota_t = cpool.tile([P, tt], F32)
            nc.sync.dma_start(iota_t[:], iot.ap())
            icnt_t = cpool.tile([K, rpc, 1], F32)
            nc.sync.dma_start(icnt_t[:], icnt.ap())

            # ---- constants ----
            id16 = cpool.tile([P, P], F16)
            make_identity(nc, id16[:])
            ones_k = cpool.tile([1, K], F16)
            nc.vector.memset(ones_k[:], 1.0)
            ones_p = cpool.tile([1, P], F32)
            nc.vector.memset(ones_p[:], 1.0)
            b16 = cpool.tile([1, H], F16)
            nc.vector.tensor_copy(b16[:], b32[:])

            # broadcast segment boundaries to all 128 partitions via PE
            psxb = ptr_pool.tile([P, rpc * (K + 1)], F32, tag="ptr")
            nc.tensor.matmul(psxb[:], ones_p[:], sx1[:], start=True, stop=True)
            sxb = cpool.tile([P, rpc, K + 1], F32)
            nc.vector.tensor_copy(sxb[:], psxb[:].rearrange("p (r k) -> p r k", r=rpc))

            hid_v = hid.ap().rearrange("(r n p) h -> p r n h", r=rpc, p=P)

            def early(r):
                # ---- early dense work: bias + tab @ Wt.T into pout PSUM ----
                pout = pout_pool.tile([K, H], F32)
                nc.tensor.matmul(
                    pout[:, 0:512], ones_k[:], b16[:, 0:512], start=True, stop=False
                )
                nc.tensor.matmul(
                    pout[:, 512:H], ones_k[:], b16[:, 512:H], start=True, stop=False
                )
                xTt = tpool.tile([P, HT, K], F16, tag="xTt")
                for j in range(HT):
                    ptr1 = ptr_pool.tile([P, K], F16, tag="ptr")
                    nc.tensor.transpose(
                        out=ptr1[:],
                        in_=tabs[:, r, j * P : (j + 1) * P],
                        identity=id16[0:K, 0:K],
                    )
                    nc.vector.tensor_copy(xTt[:, j, :], ptr1[:])
                    nc.tensor.matmul(
                        pout[:, 0:512], xTt[:, j, :], wtt_t[:, j, 0:512],
                        start=False, stop=False,
                    )
                    nc.tensor.matmul(
                        pout[:, 512:H], xTt[:, j, :], wtt_t[:, j, 512:H],
                        start=False, stop=False,
                    )
                return pout

            row_seq = [r for _ in range(repeat) for r in range(rows_used)]
            pout_next = early(row_seq[0])
            for ridx, r in enumerate(row_seq):
                last = ridx == len(row_seq) - 1
                pout = pout_next

                # ---- segment sums into PSUM [K, H] ----
                pseg = pseg_pool.tile([K, H], F32)
                schedule = [chunk] * (tt // chunk)
                if last and taper:
                    # taper the final transfers so the closing tail starts
                    # after a short hold and the PE keeps pace with the DMA
                    ntap = sum(taper)
                    assert ntap % chunk == 0
                    schedule = schedule[: -(ntap // chunk)] + list(taper)
                t0 = 0
                for nch in schedule:
                    hbuf = hpool.tile([P, chunk, H], F32R, tag="hbuf")
                    nc.sync.dma_start(hbuf[:, 0:nch, :], hid_v[:, r, t0 : t0 + nch, :])
                    for i in range(nch):
                        t = t0 + i
                        ge = mpool.tile([P, K + 1], F32, tag="ge")
                        nc.vector.tensor_scalar(
                            ge[:],
                            sxb[:, r, :],
                            iota_t[:, t : t + 1],
                            None,
                            OP.is_le,
                        )
                        m01 = mpool.tile([P, K], F32R, tag="m01")
                        nc.vector.tensor_tensor(
                            out=m01[:],
                            in0=ge[:, 0:K],
                            in1=ge[:, 1 : K + 1],
                            op=OP.subtract,
                        )
                        nc.tensor.matmul(
                            pseg[:, 0:512],
                            m01[:],
                            hbuf[:, i, 0:512],
                            start=(t == 0),
                            stop=(t == tt - 1),
                        )
                        nc.tensor.matmul(
                            pseg[:, 512:H],
                            m01[:],
                            hbuf[:, i, 512:H],
                            start=(t == 0),
                            stop=(t == tt - 1),
                        )
                    t0 += nch

                # the next row's early dense work is issued here — ahead of
                # this row's tail — so the PE absorbs it during this row's
                # stream window instead of right before the final chunks
                if not last:
                    pout_next = early(row_seq[ridx + 1])

                # ---- tail: segment mean, transpose, W_dense, tanh, store ----
                segs = spool.tile([K, H], F16, tag="segs")
                # halves on DVE and ACT concurrently (Copy shares the loaded
                # ACT table with Tanh, so no table-reload penalty)
                nc.vector.tensor_scalar(
                    segs[:, 0 : H // 2], pseg[:, 0 : H // 2], icnt_t[:, r, :],
                    None, OP.mult,
                )
                nc.scalar.activation(
                    out=segs[:, H // 2 : H],
                    in_=pseg[:, H // 2 : H],
                    func=mybir.ActivationFunctionType.Copy,
                    scale=icnt_t[:, r, :],
                )
                for j in range(HT):
                    ptr2 = ptr_pool.tile([P, K], F16, tag="ptr")
                    nc.tensor.transpose(
                        out=ptr2[:],
                        in_=segs[:, j * P : (j + 1) * P],
                        identity=id16[0:K, 0:K],
                    )
                    xTs = tpool.tile([P, K], F16, tag=f"xTs{j}")
                    if j % 2 == 0:
                        nc.vector.tensor_copy(xTs[:], ptr2[:])
                    else:
                        nc.scalar.activation(
                            out=xTs[:], in_=ptr2[:],
                            func=mybir.ActivationFunctionType.Copy,
                        )
                    nc.tensor.matmul(
                        pout[:, 0:512], xTs[:], wdt_t[:, j, 0:512],
                        start=False, stop=(j == HT - 1),
                    )
                    nc.tensor.matmul(
                        pout[:, 512:H], xTs[:], wdt_t[:, j, 512:H],
                        start=False, stop=(j == HT - 1),
                    )

                fin = spool.tile([K, H], F32, tag="fin")
                nc.scalar.activation(
                    out=fin[:],
                    in_=pout[:],
                    func=mybir.ActivationFunctionType.Tanh,
                )
                # the last store rides the (idle) sync queue: its descriptor
                # path is ~240ns shorter than the scalar queue's. Earlier
                # rows stay on the scalar queue so their data-wait can never
                # stall upcoming hidden-chunk descriptor generation.
                (nc.sync if last else nc.scalar).dma_start(out.ap()[r], fin[:])

    nc.compile()
    return nc


def prep_inputs(hidden_states, W_dense, b_dense, W_tab, b_tab, cls_indexes,
                table_length, s=S, rpc=RPC, ncores=NCORES):
    """Host-side index prep + per-core sharding. Returns in_maps."""
    hs = np.ascontiguousarray(np.asarray(hidden_states, dtype=np.float32))
    b = hs.shape[0]
    pos = np.asarray(cls_indexes)[:, 1].reshape(b, K).astype(np.int64)
    L = np.asarray(table_length).astype(np.int64)
    tt = s // P

    # sx[b, k] = min(pos_k, L) for k < K; sx[b, K] = L
    sx_all = np.minimum(pos, L[:, None]).astype(np.float32)
    sx_all = np.concatenate([sx_all, L[:, None].astype(np.float32)], axis=1)  # [b, K+1]
    cnt = sx_all[:, 1:] - sx_all[:, :-1]
    inv_cnt = np.where(cnt > 0, 1.0 / np.maximum(cnt, 1.0), 0.0).astype(np.float32)

    wdt = np.ascontiguousarray(np.asarray(W_dense, dtype=np.float32).T.astype(np.float16))
    wtt = np.ascontiguousarray(np.asarray(W_tab, dtype=np.float32).T.astype(np.float16))
    bia = np.ascontiguousarray(
        (np.asarray(b_dense, dtype=np.float32)
         + np.asarray(b_tab, dtype=np.float32))[None, :]
    )
    iot = (np.arange(P, dtype=np.float32)[:, None]
           + P * np.arange(tt, dtype=np.float32)[None, :])
    iot = np.ascontiguousarray(iot)

    in_maps = []
    for c in range(ncores):
        sxs_c = np.ascontiguousarray(
            sx_all[c * rpc:(c + 1) * rpc].reshape(1, rpc * (K + 1))
        )
        icnt_c = np.ascontiguousarray(
            inv_cnt[c * rpc:(c + 1) * rpc, :, None].transpose(1, 0, 2)
        )
        gidx_c = np.ascontiguousarray(
            (pos[c * rpc:(c + 1) * rpc] + (np.arange(rpc) * s)[:, None])
            .astype(np.int32)[:, :, None].transpose(1, 0, 2)
        )
        in_maps.append({
            "hid": hs[c * rpc:(c + 1) * rpc].reshape(rpc * s, H),
            "sxs": sxs_c,
            "icnt": icnt_c,
            "gidx": gidx_c,
            "wdt": wdt,
            "wtt": wtt,
            "bia": bia,
            "iot": iot,
        })
    return in_maps


_NC_CACHE = {}


def _get_nc():
    if "nc" not in _NC_CACHE:
        _NC_CACHE["nc"] = build_nc()
    return _NC_CACHE["nc"]


def run(inputs, trace=False):
    """Run on 8 cores; returns (full_output, BassKernelResults)."""
    import os

    nc = _get_nc()
    in_maps = prep_inputs(**inputs)
    prev = os.environ.get("BASS_NEVER_TRACE")
    if not trace:
        os.environ["BASS_NEVER_TRACE"] = "1"
    try:
        res = run_bass_kernel_spmd(
            nc, in_maps, core_ids=list(range(NCORES)), trace=trace
        )
    finally:
        if not trace:
            if prev is None:
                os.environ.pop("BASS_NEVER_TRACE", None)
            else:
                os.environ["BASS_NEVER_TRACE"] = prev
    outs = [res.results[c]["out"].reshape(RPC * K, H) for c in range(NCORES)]
    return np.concatenate(outs, axis=0), res


def kernel(**inputs) -> np.ndarray:
    out, _ = run(inputs, trace=False)
    return out


def bench(inputs, iters=20):
    """Time the on-device NEFF execution: inputs staged to the 8 devices
    once, then `iters` pipelined executes. Returns (output, secs_per_iter)."""
    nc = _get_nc()
    in_maps = prep_inputs(**inputs)
    rets, dt, dt_ser = pjrt_bench(nc, in_maps, iters)
    final = np.asarray(rets[0]).reshape(NCORES, RPC * K, H).reshape(B * K, H)
    return final, dt, dt_ser


def pjrt_bench(nc, in_maps, iters=20, ncores=NCORES):
    """Generic: jit+shard a Bass module on `ncores` devices, stage inputs,
    time pipelined and serialized executes. Returns (concat_outs, dt, dt_ser)."""
    rets, timeit = make_runner(nc, in_maps, ncores)
    dt = min(timeit(iters) for _ in range(3))
    dt_ser = dt
    return rets, dt, dt_ser


def make_runner(nc, in_maps, ncores=NCORES):
    """Stage a Bass module + inputs on the devices; return (outputs,
    timeit(iters) -> secs/iter for pipelined executes)."""
    import time

    import jax
    from jax.sharding import Mesh, NamedSharding, PartitionSpec
    from jax.experimental.shard_map import shard_map

    from concourse import bass2jax

    bass2jax.install_neuronx_cc_hook()

    partition_name = nc.partition_id_tensor.name if nc.partition_id_tensor else None
    in_names, out_names, out_avals = [], [], []
    for alloc in nc.m.functions[0].allocations:
        if not isinstance(alloc, mybir.MemoryLocationSet):
            continue
        name = alloc.memorylocations[0].name
        if alloc.kind == "ExternalInput":
            if name != partition_name:
                in_names.append(name)
        elif alloc.kind == "ExternalOutput":
            out_names.append(name)
            out_avals.append(
                jax.core.ShapedArray(
                    tuple(alloc.tensor_shape), mybir.dt.np(alloc.dtype)
                )
            )
    n_params = len(in_names)
    all_names = tuple(in_names) + tuple(out_names)
    if partition_name is not None:
        all_names = all_names + (partition_name,)

    def _body(*args):
        operands = list(args)
        if partition_name is not None:
            operands.append(bass2jax.partition_id_tensor())
        outs = bass2jax._bass_exec_p.bind(
            *operands,
            out_avals=tuple(out_avals),
            in_names=all_names,
            out_names=tuple(out_names),
            lowering_input_output_aliases=(),
            sim_require_finite=True,
            sim_require_nnan=True,
            nc=nc,
        )
        return tuple(outs)

    devices = jax.devices()[:ncores]
    mesh = Mesh(np.asarray(devices), ("core",))
    spec = PartitionSpec("core")
    nspecs = n_params + len(out_names)
    sharded = jax.jit(
        shard_map(
            _body,
            mesh=mesh,
            in_specs=(spec,) * nspecs,
            out_specs=(spec,) * len(out_names),
            check_rep=False,
        ),
        keep_unused=True,
    )
    sh = NamedSharding(mesh, spec)
    concat_in = [
        jax.device_put(
            np.concatenate([np.asarray(in_maps[c][n]) for c in range(ncores)], 0), sh
        )
        for n in in_names
    ]
    concat_zero = [
        jax.device_put(
            np.zeros((ncores * a.shape[0], *a.shape[1:]), a.dtype), sh
        )
        for a in out_avals
    ]

    out = sharded(*concat_in, *concat_zero)
    jax.block_until_ready(out)

    def timeit(iters):
        t0 = time.perf_counter()
        rets = [sharded(*concat_in, *concat_zero) for _ in range(iters)]
        jax.block_until_ready(rets)
        return (time.perf_counter() - t0) / iters

    return out, timeit



# revision 6
# speedup vs baseline: 1.0178x; 1.0145x over previous
"""BertMultiPooler (segment_reduce) Trainium2 Bass kernel, v2.

out[b*K+k] = tanh( segmean(hidden[b], seg k) @ Wd.T + bd
                   + hidden[b, pos[b,k]] @ Wt.T + bt )

Strategy (data-parallel over batch, 8 cores x 4 rows). The cost model's
binding constraint is the serial per-core DMA stream (~360 B/ns), so the
kernel is organized to keep that stream saturated end to end and to
minimize total bytes moved:
  - hidden streamed fp32 via HWDGE directly into float32r tiles: the PE
    consumes fp32r at 1 cycle/column (output free size >= 256), so the
    fp32->fp16 cast stage of v1 (83us of ACT time + pipeline stalls) is
    gone entirely.
  - Segment sums via one-hot membership matmul: for each 128-token tile,
    M[t, k] = [t >= s_k] - [t >= s_{k+1}] built on DVE in fp32r, then
    PE-matmul M.T @ hidden_tile accumulating into PSUM [64, 768].
  - Weights are cast to fp16 on the host (halves their DMA bytes); the
    combined bias row enters the dense PSUM accumulation via a
    ones-vector matmul, removing the bias DMA + DVE add.
  - CLS rows gathered with fp16 cast-DMA up front; their transposes and
    the W_tab dense matmuls run during the bulk stream, so the per-row
    tail is only: segment-mean scale, 6 transposes, 6 W_dense matmuls,
    tanh, store. The last row's transfers taper down (4,2,1,1 tiles) and
    its tab-dense work is interleaved between them as PE ramp filler, so
    the closing tail runs at full PE clock right after the last byte.
"""

import numpy as np
from contextlib import ExitStack

import concourse.bass as bass
import concourse.bacc as bacc
import concourse.tile as tile
from concourse import mybir
from concourse.bass_utils import run_bass_kernel_spmd
from concourse.masks import make_identity

B, S, H, K = 32, 4096, 768, 64
NCORES = 8
RPC = B // NCORES  # batch rows per core
P = 128
HT = H // P        # 6 h-tiles
F32 = mybir.dt.float32
F32R = mybir.dt.float32r
F16 = mybir.dt.float16
I32 = mybir.dt.int32
OP = mybir.AluOpType


def build_nc(s=S, rpc=RPC, chunk=8, hbufs=5, rows_used=None, repeat=1,
             taper=(4, 2, 1, 1)):
    """Build the per-core Bass module. Each core gets `rpc` batch rows of
    `s` tokens each. rows_used (for benching): only process that many rows."""
    tt = s // P  # token tiles per row
    assert tt % chunk == 0
    if rows_used is None:
        rows_used = rpc

    nc = bacc.Bacc("TRN2", target_bir_lowering=False, debug=False)

    # hidden is declared float32r (same bits as the fp32 input) so the PE
    # can consume the DMA-written tiles in fast-fp32 mode directly.
    hid = nc.dram_tensor("hid", [rpc * s, H], F32R, kind="ExternalInput")
    # sxs[r*(K+1)+k] = min(pos[r,k], L) for k < K, = L for k = K
    sxs = nc.dram_tensor("sxs", [1, rpc * (K + 1)], F32, kind="ExternalInput")
    icnt = nc.dram_tensor("icnt", [K, rpc, 1], F32, kind="ExternalInput")
    gidx = nc.dram_tensor("gidx", [K, rpc, 1], I32, kind="ExternalInput")
    wdt = nc.dram_tensor("wdt", [H, H], F16, kind="ExternalInput")  # W_dense.T
    wtt = nc.dram_tensor("wtt", [H, H], F16, kind="ExternalInput")  # W_tab.T
    bia = nc.dram_tensor("bia", [1, H], F32, kind="ExternalInput")  # bd+bt
    iot = nc.dram_tensor("iot", [P, tt], F32, kind="ExternalInput")  # iot[p,i]=p+128*i
    out = nc.dram_tensor("out", [rpc, K, H], F32, kind="ExternalOutput")

    with tile.TileContext(nc) as tc:
        with ExitStack() as ctx:
            cpool = ctx.enter_context(tc.tile_pool(name="const", bufs=1))
            hpool = ctx.enter_context(tc.tile_pool(name="hpool", bufs=hbufs))
            mpool = ctx.enter_context(tc.tile_pool(name="mpool", bufs=12))
            spool = ctx.enter_context(tc.tile_pool(name="spool", bufs=2))
            tpool = ctx.enter_context(tc.tile_pool(name="tpool", bufs=2))
            # PSUM banks: pseg 1x2 + pout 2x2 + ptr 2x1 = 8 exactly. pseg
            # bufs=1 is safe: the next row's first matmul sits behind this
            # row's whole tail in the in-order PE queue, so the segment-mean
            # read has long drained the bank by then.
            pseg_pool = ctx.enter_context(
                tc.tile_pool(name="pseg", bufs=1, space="PSUM")
            )
            pout_pool = ctx.enter_context(
                tc.tile_pool(name="pout", bufs=2, space="PSUM")
            )
            ptr_pool = ctx.enter_context(tc.tile_pool(name="ptr", bufs=2, space="PSUM"))

            # DMA-engine grants are FIFO by descriptor-ready time, so issue
            # order here is a schedule: gidx rides the scalar queue first (the
            # CLS gathers' prep blocks on its data), the gathers enter the
            # FIFO next, and the small consts sit on the sync queue between
            # the weights and the hidden chunks — their descriptor-gen time
            # delays the chunks' FIFO entry just enough for the gathers to
            # land before the third 17us chunk hold.
            gidx_t = cpool.tile([K, rpc, 1], I32)
            nc.scalar.dma_start(gidx_t[:], gidx.ap())
            tabs = cpool.tile([K, rpc, H], F16)
            for r in range(rpc):
                nc.gpsimd.indirect_dma_start(
                    out=tabs[:, r, :],
                    out_offset=None,
                    in_=hid.ap(),
                    in_offset=bass.IndirectOffsetOnAxis(ap=gidx_t[:, r, :], axis=0),
                )

            wdt_t = cpool.tile([P, HT, H], F16)
            nc.sync.dma_start(wdt_t[:], wdt.ap().rearrange("(j p) h -> p j h", p=P))
            wtt_t = cpool.tile([P, HT, H], F16)
            nc.sync.dma_start(wtt_t[:], wtt.ap().rearrange("(j p) h -> p j h", p=P))
            sx1 = cpool.tile([1, rpc * (K + 1)], F32)
            nc.sync.dma_start(sx1[:], sxs.ap())
            b32 = cpool.tile([1, H], F32)
            nc.sync.dma_start(b32[:], bia.ap())
            iota_t = cpool.tile([P, tt], F32)
            nc.sync.dma_start(iota_t[:], iot.ap())
            icnt_t = cpool.tile([K, rpc, 1], F32)
            nc.sync.dma_start(icnt_t[:], icnt.ap())

            # ---- constants ----
            id16 = cpool.tile([P, P], F16)
            make_identity(nc, id16[:])
            ones_k = cpool.tile([1, K], F16)
            nc.vector.memset(ones_k[:], 1.0)
            ones_p = cpool.tile([1, P], F32)
            nc.vector.memset(ones_p[:], 1.0)
            b16 = cpool.tile([1, H], F16)
            nc.vector.tensor_copy(b16[:], b32[:])

            # broadcast segment boundaries to all 128 partitions via PE
            psxb = ptr_pool.tile([P, rpc * (K + 1)], F32, tag="ptr")
            nc.tensor.matmul(psxb[:], ones_p[:], sx1[:], start=True, stop=True)
            sxb = cpool.tile([P, rpc, K + 1], F32)
            nc.vector.tensor_copy(sxb[:], psxb[:].rearrange("p (r k) -> p r k", r=rpc))

            hid_v = hid.ap().rearrange("(r n p) h -> p r n h", r=rpc, p=P)

            def early(r, defer_tab=False):
                # ---- early dense work: bias + tab @ Wt.T into pout PSUM ----
                pout = pout_pool.tile([K, H], F32)
                nc.tensor.matmul(
                    pout[:, 0:512], ones_k[:], b16[:, 0:512], start=True, stop=False
                )
                nc.tensor.matmul(
                    pout[:, 512:H], ones_k[:], b16[:, 512:H], start=True, stop=False
                )
                xTt = tpool.tile([P, HT, K], F16, tag="xTt")

                def tab_unit(j):
                    ptr1 = ptr_pool.tile([P, K], F16, tag="ptr", name="ptr1")
                    nc.tensor.transpose(
                        out=ptr1[:],
                        in_=tabs[:, r, j * P : (j + 1) * P],
                        identity=id16[0:K, 0:K],
                    )
                    nc.vector.tensor_copy(xTt[:, j, :], ptr1[:])
                    nc.tensor.matmul(
                        pout[:, 0:512], xTt[:, j, :], wtt_t[:, j, 0:512],
                        start=False, stop=False,
                    )
                    nc.tensor.matmul(
                        pout[:, 512:H], xTt[:, j, :], wtt_t[:, j, 512:H],
                        start=False, stop=False,
                    )

                if defer_tab:
                    # the last row's tab work is interleaved between its
                    # end-game chunks as PE ramp filler (see chunk loop)
                    return pout, [lambda j=j: tab_unit(j) for j in range(HT)]
                for j in range(HT):
                    tab_unit(j)
                return pout, []

            row_seq = [r for _ in range(repeat) for r in range(rows_used)]
            pout_next, fillers_next = early(row_seq[0], defer_tab=len(row_seq) == 1)
            for ridx, r in enumerate(row_seq):
                last = ridx == len(row_seq) - 1
                pout, fillers = pout_next, fillers_next

                # ---- segment sums into PSUM [K, H] ----
                pseg = pseg_pool.tile([K, H], F32)
                schedule = [chunk] * (tt // chunk)
                if last and taper:
                    # taper the final transfers so the closing tail starts
                    # after a short hold and the PE keeps pace with the DMA
                    ntap = sum(taper)
                    assert ntap % chunk == 0
                    schedule = schedule[: -(ntap // chunk)] + list(taper)
                t0 = 0
                for ci, nch in enumerate(schedule):
                    hbuf = hpool.tile([P, chunk, H], F32R, tag="hbuf")
                    nc.sync.dma_start(hbuf[:, 0:nch, :], hid_v[:, r, t0 : t0 + nch, :])
                    for i in range(nch):
                        t = t0 + i
                        ge = mpool.tile([P, K + 1], F32, tag="ge")
                        nc.vector.tensor_scalar(
                            ge[:],
                            sxb[:, r, :],
                            iota_t[:, t : t + 1],
                            None,
                            OP.is_le,
                        )
                        m01 = mpool.tile([P, K], F32R, tag="m01")
                        nc.vector.tensor_tensor(
                            out=m01[:],
                            in0=ge[:, 0:K],
                            in1=ge[:, 1 : K + 1],
                            op=OP.subtract,
                        )
                        nc.tensor.matmul(
                            pseg[:, 0:512],
                            m01[:],
                            hbuf[:, i, 0:512],
                            start=(t == 0),
                            stop=(t == tt - 1),
                        )
                        nc.tensor.matmul(
                            pseg[:, 512:H],
                            m01[:],
                            hbuf[:, i, 512:H],
                            start=(t == 0),
                            stop=(t == tt - 1),
                        )
                    t0 += nch
                    # deferred tab units keep the PE ramped between the
                    # end-game transfers
                    if fillers and ci >= 1:
                        if ci == len(schedule) - 1:
                            while fillers:
                                fillers.pop(0)()
                        else:
                            fillers.pop(0)()

                while fillers:  # safety for very short schedules
                    fillers.pop(0)()

                # the next row's early dense work is issued here — ahead of
                # this row's tail — so the PE absorbs it during this row's
                # stream window instead of right before the final chunks
                if not last:
                    pout_next, fillers_next = early(
                        row_seq[ridx + 1], defer_tab=ridx + 1 == len(row_seq) - 1
                    )

                # ---- tail: segment mean, transpose, W_dense, tanh, store ----
                segs = spool.tile([K, H], F16, tag="segs")
                # two DVE halves; ACT handles alternate transpose copies below
                nc.vector.tensor_scalar(
                    segs[:, 0 : H // 2], pseg[:, 0 : H // 2], icnt_t[:, r, :],
                    None, OP.mult,
                )
                nc.vector.tensor_scalar(
                    segs[:, H // 2 : H], pseg[:, H // 2 : H], icnt_t[:, r, :],
                    None, OP.mult,
                )
                for j in range(HT):
                    ptr2 = ptr_pool.tile([P, K], F16, tag="ptr")
                    nc.tensor.transpose(
                        out=ptr2[:],
                        in_=segs[:, j * P : (j + 1) * P],
                        identity=id16[0:K, 0:K],
                    )
                    xTs = tpool.tile([P, K], F16, tag=f"xTs{j}")
                    if j % 2 == 0:
                        nc.vector.tensor_copy(xTs[:], ptr2[:])
                    else:
                        nc.scalar.activation(
                            out=xTs[:], in_=ptr2[:],
                            func=mybir.ActivationFunctionType.Copy,
                        )
                    nc.tensor.matmul(
                        pout[:, 0:512], xTs[:], wdt_t[:, j, 0:512],
                        start=False, stop=(j == HT - 1),
                    )
                    nc.tensor.matmul(
                        pout[:, 512:H], xTs[:], wdt_t[:, j, 512:H],
                        start=False, stop=(j == HT - 1),
                    )

                fin = spool.tile([K, H], F32, tag="fin")
                nc.scalar.activation(
                    out=fin[:],
                    in_=pout[:],
                    func=mybir.ActivationFunctionType.Tanh,
                )
                # the last store rides the (idle) sync queue: its descriptor
                # path is ~240ns shorter than the scalar queue's. Earlier
                # rows stay on the scalar queue so their data-wait can never
                # stall upcoming hidden-chunk descriptor generation.
                (nc.sync if last else nc.scalar).dma_start(out.ap()[r], fin[:])

    nc.compile()
    return nc


def prep_inputs(hidden_states, W_dense, b_dense, W_tab, b_tab, cls_indexes,
                table_length, s=S, rpc=RPC, ncores=NCORES):
    """Host-side index prep + per-core sharding. Returns in_maps."""
    hs = np.ascontiguousarray(np.asarray(hidden_states, dtype=np.float32))
    b = hs.shape[0]
    pos = np.asarray(cls_indexes)[:, 1].reshape(b, K).astype(np.int64)
    L = np.asarray(table_length).astype(np.int64)
    tt = s // P

    # sx[b, k] = min(pos_k, L) for k < K; sx[b, K] = L
    sx_all = np.minimum(pos, L[:, None]).astype(np.float32)
    sx_all = np.concatenate([sx_all, L[:, None].astype(np.float32)], axis=1)  # [b, K+1]
    cnt = sx_all[:, 1:] - sx_all[:, :-1]
    inv_cnt = np.where(cnt > 0, 1.0 / np.maximum(cnt, 1.0), 0.0).astype(np.float32)

    wdt = np.ascontiguousarray(np.asarray(W_dense, dtype=np.float32).T.astype(np.float16))
    wtt = np.ascontiguousarray(np.asarray(W_tab, dtype=np.float32).T.astype(np.float16))
    bia = np.ascontiguousarray(
        (np.asarray(b_dense, dtype=np.float32)
         + np.asarray(b_tab, dtype=np.float32))[None, :]
    )
    iot = (np.arange(P, dtype=np.float32)[:, None]
           + P * np.arange(tt, dtype=np.float32)[None, :])
    iot = np.ascontiguousarray(iot)

    in_maps = []
    for c in range(ncores):
        sxs_c = np.ascontiguousarray(
            sx_all[c * rpc:(c + 1) * rpc].reshape(1, rpc * (K + 1))
        )
        icnt_c = np.ascontiguousarray(
            inv_cnt[c * rpc:(c + 1) * rpc, :, None].transpose(1, 0, 2)
        )
        gidx_c = np.ascontiguousarray(
            (pos[c * rpc:(c + 1) * rpc] + (np.arange(rpc) * s)[:, None])
            .astype(np.int32)[:, :, None].transpose(1, 0, 2)
        )
        in_maps.append({
            "hid": hs[c * rpc:(c + 1) * rpc].reshape(rpc * s, H),
            "sxs": sxs_c,
            "icnt": icnt_c,
            "gidx": gidx_c,
            "wdt": wdt,
            "wtt": wtt,
            "bia": bia,
            "iot": iot,
        })
    return in_maps


_NC_CACHE = {}


def _get_nc():
    if "nc" not in _NC_CACHE:
        _NC_CACHE["nc"] = build_nc()
    return _NC_CACHE["nc"]


def run(inputs, trace=False):
    """Run on 8 cores; returns (full_output, BassKernelResults)."""
    import os

    nc = _get_nc()
    in_maps = prep_inputs(**inputs)
    prev = os.environ.get("BASS_NEVER_TRACE")
    if not trace:
        os.environ["BASS_NEVER_TRACE"] = "1"
    try:
        res = run_bass_kernel_spmd(
            nc, in_maps, core_ids=list(range(NCORES)), trace=trace
        )
    finally:
        if not trace:
            if prev is None:
                os.environ.pop("BASS_NEVER_TRACE", None)
            else:
                os.environ["BASS_NEVER_TRACE"] = prev
    outs = [res.results[c]["out"].reshape(RPC * K, H) for c in range(NCORES)]
    return np.concatenate(outs, axis=0), res


def kernel(**inputs) -> np.ndarray:
    out, _ = run(inputs, trace=False)
    return out


def bench(inputs, iters=20):
    """Time the on-device NEFF execution: inputs staged to the 8 devices
    once, then `iters` pipelined executes. Returns (output, secs_per_iter)."""
    nc = _get_nc()
    in_maps = prep_inputs(**inputs)
    rets, dt, dt_ser = pjrt_bench(nc, in_maps, iters)
    final = np.asarray(rets[0]).reshape(NCORES, RPC * K, H).reshape(B * K, H)
    return final, dt, dt_ser


def pjrt_bench(nc, in_maps, iters=20, ncores=NCORES):
    """Generic: jit+shard a Bass module on `ncores` devices, stage inputs,
    time pipelined and serialized executes. Returns (concat_outs, dt, dt_ser)."""
    rets, timeit = make_runner(nc, in_maps, ncores)
    dt = min(timeit(iters) for _ in range(3))
    dt_ser = dt
    return rets, dt, dt_ser


def make_runner(nc, in_maps, ncores=NCORES):
    """Stage a Bass module + inputs on the devices; return (outputs,
    timeit(iters) -> secs/iter for pipelined executes)."""
    import time

    import jax
    from jax.sharding import Mesh, NamedSharding, PartitionSpec
    from jax.experimental.shard_map import shard_map

    from concourse import bass2jax

    bass2jax.install_neuronx_cc_hook()

    partition_name = nc.partition_id_tensor.name if nc.partition_id_tensor else None
    in_names, out_names, out_avals = [], [], []
    for alloc in nc.m.functions[0].allocations:
        if not isinstance(alloc, mybir.MemoryLocationSet):
            continue
        name = alloc.memorylocations[0].name
        if alloc.kind == "ExternalInput":
            if name != partition_name:
                in_names.append(name)
        elif alloc.kind == "ExternalOutput":
            out_names.append(name)
            out_avals.append(
                jax.core.ShapedArray(
                    tuple(alloc.tensor_shape), mybir.dt.np(alloc.dtype)
                )
            )
    n_params = len(in_names)
    all_names = tuple(in_names) + tuple(out_names)
    if partition_name is not None:
        all_names = all_names + (partition_name,)

    def _body(*args):
        operands = list(args)
        if partition_name is not None:
            operands.append(bass2jax.partition_id_tensor())
        outs = bass2jax._bass_exec_p.bind(
            *operands,
            out_avals=tuple(out_avals),
            in_names=all_names,
            out_names=tuple(out_names),
            lowering_input_output_aliases=(),
            sim_require_finite=True,
            sim_require_nnan=True,
            nc=nc,
        )
        return tuple(outs)

    devices = jax.devices()[:ncores]
    mesh = Mesh(np.asarray(devices), ("core",))
    spec = PartitionSpec("core")
    nspecs = n_params + len(out_names)
    sharded = jax.jit(
        shard_map(
            _body,
            mesh=mesh,
            in_specs=(spec,) * nspecs,
            out_specs=(spec,) * len(out_names),
            check_rep=False,
        ),
        keep_unused=True,
    )
    sh = NamedSharding(mesh, spec)
    concat_in = [
        jax.device_put(
            np.concatenate([np.asarray(in_maps[c][n]) for c in range(ncores)], 0), sh
        )
        for n in in_names
    ]
    concat_zero = [
        jax.device_put(
            np.zeros((ncores * a.shape[0], *a.shape[1:]), a.dtype), sh
        )
        for a in out_avals
    ]

    out = sharded(*concat_in, *concat_zero)
    jax.block_until_ready(out)

    def timeit(iters):
        t0 = time.perf_counter()
        rets = [sharded(*concat_in, *concat_zero) for _ in range(iters)]
        jax.block_until_ready(rets)
        return (time.perf_counter() - t0) / iters

    return out, timeit



# revision 7
# speedup vs baseline: 1.0230x; 1.0051x over previous
"""BertMultiPooler (segment_reduce) Trainium2 Bass kernel, v2.

out[b*K+k] = tanh( segmean(hidden[b], seg k) @ Wd.T + bd
                   + hidden[b, pos[b,k]] @ Wt.T + bt )

Strategy (data-parallel over batch, 8 cores x 4 rows). The cost model's
binding constraint is the serial per-core DMA stream (~360 B/ns), so the
kernel is organized to keep that stream saturated end to end and to
minimize total bytes moved:
  - hidden streamed fp32 via HWDGE directly into float32r tiles: the PE
    consumes fp32r at 1 cycle/column (output free size >= 256), so the
    fp32->fp16 cast stage of v1 (83us of ACT time + pipeline stalls) is
    gone entirely.
  - Segment sums via one-hot membership matmul: for each 128-token tile,
    M[t, k] = [t >= s_k] - [t >= s_{k+1}] built on DVE in fp32r, then
    PE-matmul M.T @ hidden_tile accumulating into PSUM [64, 768].
  - Weights are cast to fp16 on the host (halves their DMA bytes); the
    combined bias row enters the dense PSUM accumulation via a
    ones-vector matmul, removing the bias DMA + DVE add.
  - CLS rows gathered with fp16 cast-DMA up front; their transposes and
    the W_tab dense matmuls run during the bulk stream, so the per-row
    tail is only: segment-mean scale, 6 transposes, 6 W_dense matmuls,
    tanh, store. The last row's transfers taper down (4,2,1,1 tiles) and
    its tab-dense work is interleaved between them as PE ramp filler, so
    the closing tail runs at full PE clock right after the last byte.
"""

import numpy as np
from contextlib import ExitStack

import concourse.bass as bass
import concourse.bacc as bacc
import concourse.tile as tile
from concourse import mybir
from concourse.bass_utils import run_bass_kernel_spmd
from concourse.masks import make_identity

B, S, H, K = 32, 4096, 768, 64
NCORES = 8
RPC = B // NCORES  # batch rows per core
P = 128
HT = H // P        # 6 h-tiles
F32 = mybir.dt.float32
F32R = mybir.dt.float32r
F16 = mybir.dt.float16
I32 = mybir.dt.int32
OP = mybir.AluOpType


def build_nc(s=S, rpc=RPC, chunk=8, hbufs=5, rows_used=None, repeat=1,
             taper=(4, 2, 1, 1)):
    """Build the per-core Bass module. Each core gets `rpc` batch rows of
    `s` tokens each. rows_used (for benching): only process that many rows."""
    tt = s // P  # token tiles per row
    assert tt % chunk == 0
    if rows_used is None:
        rows_used = rpc

    nc = bacc.Bacc("TRN2", target_bir_lowering=False, debug=False)

    # hidden is declared float32r (same bits as the fp32 input) so the PE
    # can consume the DMA-written tiles in fast-fp32 mode directly.
    hid = nc.dram_tensor("hid", [rpc * s, H], F32R, kind="ExternalInput")
    # sxs[r*(K+1)+k] = min(pos[r,k], L) for k < K, = L for k = K
    sxs = nc.dram_tensor("sxs", [1, rpc * (K + 1)], F32, kind="ExternalInput")
    icnt = nc.dram_tensor("icnt", [K, rpc, 1], F32, kind="ExternalInput")
    gidx = nc.dram_tensor("gidx", [K, rpc, 1], I32, kind="ExternalInput")
    wdt = nc.dram_tensor("wdt", [H, H], F16, kind="ExternalInput")  # W_dense.T
    wtt = nc.dram_tensor("wtt", [H, H], F16, kind="ExternalInput")  # W_tab.T
    bia = nc.dram_tensor("bia", [1, H], F32, kind="ExternalInput")  # bd+bt
    iot = nc.dram_tensor("iot", [P, tt], F32, kind="ExternalInput")  # iot[p,i]=p+128*i
    # fp16 output: tanh lands in [-1,1] where fp16 keeps ~5e-4 relative
    # precision; halving the store bytes shortens the DMA stream and the
    # final hold. The host upcasts to fp32.
    out = nc.dram_tensor("out", [rpc, K, H], F16, kind="ExternalOutput")

    with tile.TileContext(nc) as tc:
        with ExitStack() as ctx:
            cpool = ctx.enter_context(tc.tile_pool(name="const", bufs=1))
            hpool = ctx.enter_context(tc.tile_pool(name="hpool", bufs=hbufs))
            mpool = ctx.enter_context(tc.tile_pool(name="mpool", bufs=12))
            spool = ctx.enter_context(tc.tile_pool(name="spool", bufs=2))
            tpool = ctx.enter_context(tc.tile_pool(name="tpool", bufs=2))
            # PSUM banks: pseg 1x2 + pout 2x2 + ptr 2x1 = 8 exactly. pseg
            # bufs=1 is safe: the next row's first matmul sits behind this
            # row's whole tail in the in-order PE queue, so the segment-mean
            # read has long drained the bank by then.
            pseg_pool = ctx.enter_context(
                tc.tile_pool(name="pseg", bufs=1, space="PSUM")
            )
            pout_pool = ctx.enter_context(
                tc.tile_pool(name="pout", bufs=2, space="PSUM")
            )
            ptr_pool = ctx.enter_context(tc.tile_pool(name="ptr", bufs=2, space="PSUM"))

            # DMA-engine grants are FIFO by descriptor-ready time, so issue
            # order here is a schedule: gidx rides the scalar queue first (the
            # CLS gathers' prep blocks on its data), the gathers enter the
            # FIFO next, and the small consts sit on the sync queue between
            # the weights and the hidden chunks — their descriptor-gen time
            # delays the chunks' FIFO entry just enough for the gathers to
            # land before the third 17us chunk hold.
            gidx_t = cpool.tile([K, rpc, 1], I32)
            nc.scalar.dma_start(gidx_t[:], gidx.ap())
            tabs = cpool.tile([K, rpc, H], F16)
            for r in range(rpc):
                nc.gpsimd.indirect_dma_start(
                    out=tabs[:, r, :],
                    out_offset=None,
                    in_=hid.ap(),
                    in_offset=bass.IndirectOffsetOnAxis(ap=gidx_t[:, r, :], axis=0),
                )

            wdt_t = cpool.tile([P, HT, H], F16)
            nc.sync.dma_start(wdt_t[:], wdt.ap().rearrange("(j p) h -> p j h", p=P))
            wtt_t = cpool.tile([P, HT, H], F16)
            nc.sync.dma_start(wtt_t[:], wtt.ap().rearrange("(j p) h -> p j h", p=P))
            sx1 = cpool.tile([1, rpc * (K + 1)], F32)
            nc.sync.dma_start(sx1[:], sxs.ap())
            b32 = cpool.tile([1, H], F32)
            nc.sync.dma_start(b32[:], bia.ap())
            iota_t = cpool.tile([P, tt], F32)
            nc.sync.dma_start(iota_t[:], iot.ap())
            icnt_t = cpool.tile([K, rpc, 1], F32)
            nc.sync.dma_start(icnt_t[:], icnt.ap())

            # ---- constants ----
            id16 = cpool.tile([P, P], F16)
            make_identity(nc, id16[:])
            ones_k = cpool.tile([1, K], F16)
            nc.vector.memset(ones_k[:], 1.0)
            ones_p = cpool.tile([1, P], F32)
            nc.vector.memset(ones_p[:], 1.0)
            b16 = cpool.tile([1, H], F16)
            nc.vector.tensor_copy(b16[:], b32[:])

            # broadcast segment boundaries to all 128 partitions via PE
            psxb = ptr_pool.tile([P, rpc * (K + 1)], F32, tag="ptr")
            nc.tensor.matmul(psxb[:], ones_p[:], sx1[:], start=True, stop=True)
            sxb = cpool.tile([P, rpc, K + 1], F32)
            nc.vector.tensor_copy(sxb[:], psxb[:].rearrange("p (r k) -> p r k", r=rpc))

            hid_v = hid.ap().rearrange("(r n p) h -> p r n h", r=rpc, p=P)

            def early(r, defer_tab=False):
                # ---- early dense work: bias + tab @ Wt.T into pout PSUM ----
                pout = pout_pool.tile([K, H], F32)
                nc.tensor.matmul(
                    pout[:, 0:512], ones_k[:], b16[:, 0:512], start=True, stop=False
                )
                nc.tensor.matmul(
                    pout[:, 512:H], ones_k[:], b16[:, 512:H], start=True, stop=False
                )
                xTt = tpool.tile([P, HT, K], F16, tag="xTt")

                def tab_unit(j):
                    ptr1 = ptr_pool.tile([P, K], F16, tag="ptr", name="ptr1")
                    nc.tensor.transpose(
                        out=ptr1[:],
                        in_=tabs[:, r, j * P : (j + 1) * P],
                        identity=id16[0:K, 0:K],
                    )
                    nc.vector.tensor_copy(xTt[:, j, :], ptr1[:])
                    nc.tensor.matmul(
                        pout[:, 0:512], xTt[:, j, :], wtt_t[:, j, 0:512],
                        start=False, stop=False,
                    )
                    nc.tensor.matmul(
                        pout[:, 512:H], xTt[:, j, :], wtt_t[:, j, 512:H],
                        start=False, stop=False,
                    )

                if defer_tab:
                    # the last row's tab work is interleaved between its
                    # end-game chunks as PE ramp filler (see chunk loop)
                    return pout, [lambda j=j: tab_unit(j) for j in range(HT)]
                for j in range(HT):
                    tab_unit(j)
                return pout, []

            row_seq = [r for _ in range(repeat) for r in range(rows_used)]
            pout_next, fillers_next = early(row_seq[0], defer_tab=len(row_seq) == 1)
            for ridx, r in enumerate(row_seq):
                last = ridx == len(row_seq) - 1
                pout, fillers = pout_next, fillers_next

                # ---- segment sums into PSUM [K, H] ----
                pseg = pseg_pool.tile([K, H], F32)
                schedule = [chunk] * (tt // chunk)
                if last and taper:
                    # taper the final transfers so the closing tail starts
                    # after a short hold and the PE keeps pace with the DMA
                    ntap = sum(taper)
                    assert ntap % chunk == 0
                    schedule = schedule[: -(ntap // chunk)] + list(taper)
                t0 = 0
                for ci, nch in enumerate(schedule):
                    hbuf = hpool.tile([P, chunk, H], F32R, tag="hbuf")
                    nc.sync.dma_start(hbuf[:, 0:nch, :], hid_v[:, r, t0 : t0 + nch, :])
                    for i in range(nch):
                        t = t0 + i
                        ge = mpool.tile([P, K + 1], F32, tag="ge")
                        nc.vector.tensor_scalar(
                            ge[:],
                            sxb[:, r, :],
                            iota_t[:, t : t + 1],
                            None,
                            OP.is_le,
                        )
                        m01 = mpool.tile([P, K], F32R, tag="m01")
                        nc.vector.tensor_tensor(
                            out=m01[:],
                            in0=ge[:, 0:K],
                            in1=ge[:, 1 : K + 1],
                            op=OP.subtract,
                        )
                        nc.tensor.matmul(
                            pseg[:, 0:512],
                            m01[:],
                            hbuf[:, i, 0:512],
                            start=(t == 0),
                            stop=(t == tt - 1),
                        )
                        nc.tensor.matmul(
                            pseg[:, 512:H],
                            m01[:],
                            hbuf[:, i, 512:H],
                            start=(t == 0),
                            stop=(t == tt - 1),
                        )
                    t0 += nch
                    # deferred tab units keep the PE ramped between the
                    # end-game transfers
                    if fillers and ci >= 1:
                        if ci == len(schedule) - 1:
                            while fillers:
                                fillers.pop(0)()
                        else:
                            fillers.pop(0)()

                while fillers:  # safety for very short schedules
                    fillers.pop(0)()

                # the next row's early dense work is issued here — ahead of
                # this row's tail — so the PE absorbs it during this row's
                # stream window instead of right before the final chunks
                if not last:
                    pout_next, fillers_next = early(
                        row_seq[ridx + 1], defer_tab=ridx + 1 == len(row_seq) - 1
                    )

                # ---- tail: segment mean, transpose, W_dense, tanh, store ----
                segs = spool.tile([K, H], F16, tag="segs")
                # two DVE halves; ACT handles alternate transpose copies below
                nc.vector.tensor_scalar(
                    segs[:, 0 : H // 2], pseg[:, 0 : H // 2], icnt_t[:, r, :],
                    None, OP.mult,
                )
                nc.vector.tensor_scalar(
                    segs[:, H // 2 : H], pseg[:, H // 2 : H], icnt_t[:, r, :],
                    None, OP.mult,
                )
                for j in range(HT):
                    ptr2 = ptr_pool.tile([P, K], F16, tag="ptr")
                    nc.tensor.transpose(
                        out=ptr2[:],
                        in_=segs[:, j * P : (j + 1) * P],
                        identity=id16[0:K, 0:K],
                    )
                    xTs = tpool.tile([P, K], F16, tag=f"xTs{j}")
                    if j % 2 == 0:
                        nc.vector.tensor_copy(xTs[:], ptr2[:])
                    else:
                        nc.scalar.activation(
                            out=xTs[:], in_=ptr2[:],
                            func=mybir.ActivationFunctionType.Copy,
                        )
                    nc.tensor.matmul(
                        pout[:, 0:512], xTs[:], wdt_t[:, j, 0:512],
                        start=False, stop=(j == HT - 1),
                    )
                    nc.tensor.matmul(
                        pout[:, 512:H], xTs[:], wdt_t[:, j, 512:H],
                        start=False, stop=(j == HT - 1),
                    )

                fin = spool.tile([K, H], F16, tag="fin")
                nc.scalar.activation(
                    out=fin[:],
                    in_=pout[:],
                    func=mybir.ActivationFunctionType.Tanh,
                )
                # the last store rides the (idle) sync queue: its descriptor
                # path is ~240ns shorter than the scalar queue's. Earlier
                # rows stay on the scalar queue so their data-wait can never
                # stall upcoming hidden-chunk descriptor generation.
                (nc.sync if last else nc.scalar).dma_start(out.ap()[r], fin[:])

    nc.compile()
    return nc


def prep_inputs(hidden_states, W_dense, b_dense, W_tab, b_tab, cls_indexes,
                table_length, s=S, rpc=RPC, ncores=NCORES):
    """Host-side index prep + per-core sharding. Returns in_maps."""
    hs = np.ascontiguousarray(np.asarray(hidden_states, dtype=np.float32))
    b = hs.shape[0]
    pos = np.asarray(cls_indexes)[:, 1].reshape(b, K).astype(np.int64)
    L = np.asarray(table_length).astype(np.int64)
    tt = s // P

    # sx[b, k] = min(pos_k, L) for k < K; sx[b, K] = L
    sx_all = np.minimum(pos, L[:, None]).astype(np.float32)
    sx_all = np.concatenate([sx_all, L[:, None].astype(np.float32)], axis=1)  # [b, K+1]
    cnt = sx_all[:, 1:] - sx_all[:, :-1]
    inv_cnt = np.where(cnt > 0, 1.0 / np.maximum(cnt, 1.0), 0.0).astype(np.float32)

    wdt = np.ascontiguousarray(np.asarray(W_dense, dtype=np.float32).T.astype(np.float16))
    wtt = np.ascontiguousarray(np.asarray(W_tab, dtype=np.float32).T.astype(np.float16))
    bia = np.ascontiguousarray(
        (np.asarray(b_dense, dtype=np.float32)
         + np.asarray(b_tab, dtype=np.float32))[None, :]
    )
    iot = (np.arange(P, dtype=np.float32)[:, None]
           + P * np.arange(tt, dtype=np.float32)[None, :])
    iot = np.ascontiguousarray(iot)

    in_maps = []
    for c in range(ncores):
        sxs_c = np.ascontiguousarray(
            sx_all[c * rpc:(c + 1) * rpc].reshape(1, rpc * (K + 1))
        )
        icnt_c = np.ascontiguousarray(
            inv_cnt[c * rpc:(c + 1) * rpc, :, None].transpose(1, 0, 2)
        )
        gidx_c = np.ascontiguousarray(
            (pos[c * rpc:(c + 1) * rpc] + (np.arange(rpc) * s)[:, None])
            .astype(np.int32)[:, :, None].transpose(1, 0, 2)
        )
        in_maps.append({
            "hid": hs[c * rpc:(c + 1) * rpc].reshape(rpc * s, H),
            "sxs": sxs_c,
            "icnt": icnt_c,
            "gidx": gidx_c,
            "wdt": wdt,
            "wtt": wtt,
            "bia": bia,
            "iot": iot,
        })
    return in_maps


_NC_CACHE = {}


def _get_nc():
    if "nc" not in _NC_CACHE:
        _NC_CACHE["nc"] = build_nc()
    return _NC_CACHE["nc"]


def run(inputs, trace=False):
    """Run on 8 cores; returns (full_output, BassKernelResults)."""
    import os

    nc = _get_nc()
    in_maps = prep_inputs(**inputs)
    prev = os.environ.get("BASS_NEVER_TRACE")
    if not trace:
        os.environ["BASS_NEVER_TRACE"] = "1"
    try:
        res = run_bass_kernel_spmd(
            nc, in_maps, core_ids=list(range(NCORES)), trace=trace
        )
    finally:
        if not trace:
            if prev is None:
                os.environ.pop("BASS_NEVER_TRACE", None)
            else:
                os.environ["BASS_NEVER_TRACE"] = prev
    outs = [res.results[c]["out"].reshape(RPC * K, H).astype(np.float32)
            for c in range(NCORES)]
    return np.concatenate(outs, axis=0), res


def kernel(**inputs) -> np.ndarray:
    out, _ = run(inputs, trace=False)
    return out


def bench(inputs, iters=20):
    """Time the on-device NEFF execution: inputs staged to the 8 devices
    once, then `iters` pipelined executes. Returns (output, secs_per_iter)."""
    nc = _get_nc()
    in_maps = prep_inputs(**inputs)
    rets, dt, dt_ser = pjrt_bench(nc, in_maps, iters)
    final = np.asarray(rets[0]).reshape(NCORES, RPC * K, H).reshape(B * K, H)
    return final, dt, dt_ser


def pjrt_bench(nc, in_maps, iters=20, ncores=NCORES):
    """Generic: jit+shard a Bass module on `ncores` devices, stage inputs,
    time pipelined and serialized executes. Returns (concat_outs, dt, dt_ser)."""
    rets, timeit = make_runner(nc, in_maps, ncores)
    dt = min(timeit(iters) for _ in range(3))
    dt_ser = dt
    return rets, dt, dt_ser


def make_runner(nc, in_maps, ncores=NCORES):
    """Stage a Bass module + inputs on the devices; return (outputs,
    timeit(iters) -> secs/iter for pipelined executes)."""
    import time

    import jax
    from jax.sharding import Mesh, NamedSharding, PartitionSpec
    from jax.experimental.shard_map import shard_map

    from concourse import bass2jax

    bass2jax.install_neuronx_cc_hook()

    partition_name = nc.partition_id_tensor.name if nc.partition_id_tensor else None
    in_names, out_names, out_avals = [], [], []
    for alloc in nc.m.functions[0].allocations:
        if not isinstance(alloc, mybir.MemoryLocationSet):
            continue
        name = alloc.memorylocations[0].name
        if alloc.kind == "ExternalInput":
            if name != partition_name:
                in_names.append(name)
        elif alloc.kind == "ExternalOutput":
            out_names.append(name)
            out_avals.append(
                jax.core.ShapedArray(
                    tuple(alloc.tensor_shape), mybir.dt.np(alloc.dtype)
                )
            )
    n_params = len(in_names)
    all_names = tuple(in_names) + tuple(out_names)
    if partition_name is not None:
        all_names = all_names + (partition_name,)

    def _body(*args):
        operands = list(args)
        if partition_name is not None:
            operands.append(bass2jax.partition_id_tensor())
        outs = bass2jax._bass_exec_p.bind(
            *operands,
            out_avals=tuple(out_avals),
            in_names=all_names,
            out_names=tuple(out_names),
            lowering_input_output_aliases=(),
            sim_require_finite=True,
            sim_require_nnan=True,
            nc=nc,
        )
        return tuple(outs)

    devices = jax.devices()[:ncores]
    mesh = Mesh(np.asarray(devices), ("core",))
    spec = PartitionSpec("core")
    nspecs = n_params + len(out_names)
    sharded = jax.jit(
        shard_map(
            _body,
            mesh=mesh,
            in_specs=(spec,) * nspecs,
            out_specs=(spec,) * len(out_names),
            check_rep=False,
        ),
        keep_unused=True,
    )
    sh = NamedSharding(mesh, spec)
    concat_in = [
        jax.device_put(
            np.concatenate([np.asarray(in_maps[c][n]) for c in range(ncores)], 0), sh
        )
        for n in in_names
    ]
    concat_zero = [
        jax.device_put(
            np.zeros((ncores * a.shape[0], *a.shape[1:]), a.dtype), sh
        )
        for a in out_avals
    ]

    out = sharded(*concat_in, *concat_zero)
    jax.block_until_ready(out)

    def timeit(iters):
        t0 = time.perf_counter()
        rets = [sharded(*concat_in, *concat_zero) for _ in range(iters)]
        jax.block_until_ready(rets)
        return (time.perf_counter() - t0) / iters

    return out, timeit

